# revision 2
# baseline (speedup 1.0000x reference)
"""Inverse 3D DWT (db4, 2 levels) Trainium2 Bass kernel, v2.

Sharding: 8 cores = 4 batch x 2 D-halves. Core (b, dh) computes the
contiguous output slab out[b, 0, 128*dh:128*(dh+1), :, :] (128x256x256).

Per level the three 1D synthesis stages run in the order D -> W -> H with
matmul orientations chosen so every stage's output lands with the next
stage's contraction dim on partitions: no PE transposes and no DRAM
round-trips. All intermediates stay in SBUF (bf16); PSUM accumulates fp32.

  level-2 (64^3 -> d-window 68 x 128 x 128):
    D: stat = stacked lo/hi input [128,64w], mov = stacked band [128,68]
    W: stat = tw2[:, :, d] (strided), mov = stacked band [128,128]
    H: stat = m2[:, :, w] (strided),  mov = same stacked band
  level-1 (68 x 128 x 128 -> 128 x 256 x 256):
    D: stat = subband [68, 128w] per h, mov = windowed band [68,128]
    W: stat = t[:, :, d] (strided),  mov = full band [128,256]
    H: stat = band slice [128,128],  mov = m1 [128, (d,w)]
"""
import sys
import numpy as np
import ml_dtypes

sys.path.insert(0, "/opt/trn_rl_repo")

_CACHE = {}
BF = ml_dtypes.bfloat16


def _band_cyclic(w, n):
    M = np.zeros((n, 2 * n), np.float32)
    for j in range(2 * n):
        k0 = (j + 3) % 2
        base = (j + 3) // 2
        for s in range(4):
            M[(base - s) % n, j] += w[k0 + 2 * s]
    return M


def _build_nc():
    import concourse.bass as bass  # noqa: F401
    import concourse.tile as tile
    import concourse.mybir as mybir
    from concourse import bacc

    f32 = mybir.dt.float32
    bf16 = mybir.dt.bfloat16

    nc = bacc.Bacc("TRN2", debug=False, num_devices=8)

    yl_s = nc.dram_tensor("yl_s", [64, 4096], f32, kind="ExternalInput")
    yh2_s = nc.dram_tensor("yh2_s", [7, 64, 4096], f32, kind="ExternalInput")
    yh1_s = nc.dram_tensor("yh1_s", [7, 68, 128, 128], f32, kind="ExternalInput")
    bD2 = nc.dram_tensor("bD2", [2, 64, 68], bf16, kind="ExternalInput")
    bW2 = nc.dram_tensor("bW2", [2, 64, 128], bf16, kind="ExternalInput")
    bD1 = nc.dram_tensor("bD1", [2, 68, 128], bf16, kind="ExternalInput")
    bc1 = nc.dram_tensor("bc1", [2, 128, 256], bf16, kind="ExternalInput")
    out = nc.dram_tensor("out", [128, 256, 256], f32, kind="ExternalOutput")

    H_CH = 8      # level-1 D h-chunk
    DC = 8        # level-1 W/H d-chunk

    # round-robin copy engines: PSUM evacuation on Act/DVE, casts mostly gpsimd
    def make_rr(pattern):
        state = [0]

        def cp(dst, src):
            e = pattern[state[0] % len(pattern)]
            state[0] += 1
            if e == 0:
                nc.scalar.copy(dst, src)
            elif e == 1:
                nc.vector.tensor_copy(dst, src)
            else:
                nc.gpsimd.tensor_copy(dst, src)

        return cp

    cp_rr = make_rr([0, 1])
    hcp_rr = make_rr([1, 0])

    # round-robin DMA issue across the two HWDGE queues (SP, Act)
    def make_dma_rr():
        state = [0]

        def dma(dst, src):
            e = state[0] % 2
            state[0] += 1
            (nc.sync if e == 0 else nc.scalar).dma_start(dst, src)

        return dma

    dma_rr = make_dma_rr()

    with tile.TileContext(nc) as tc:
        with (
            tc.tile_pool(name="pp", bufs=1) as pp,
            tc.tile_pool(name="ps", bufs=8, space="PSUM") as ps,
        ):
            # persistent tiles + constants
            lld = pp.tile([68, 128, 128], bf16, tag="lld")
            cbD2, cbW2, cbD1, cbc1 = [], [], [], []
            for s in range(2):
                tmp = pp.tile([64, 68], bf16, tag=f"cbD2_{s}", name=f"cbD2{s}")
                nc.sync.dma_start(tmp[:], bD2[s])
                cbD2.append(tmp)
                tmp = pp.tile([64, 128], bf16, tag=f"cbW2_{s}", name=f"cbW2{s}")
                nc.sync.dma_start(tmp[:], bW2[s])
                cbW2.append(tmp)
                tmp = pp.tile([68, 128], bf16, tag=f"cbD1_{s}", name=f"cbD1{s}")
                nc.sync.dma_start(tmp[:], bD1[s])
                cbD1.append(tmp)
                tmp = pp.tile([128, 256], bf16, tag=f"cbc1_{s}", name=f"cbc1{s}")
                nc.sync.dma_start(tmp[:], bc1[s])
                cbc1.append(tmp)

            # ---------------- level 2 ----------------
            with tc.tile_pool(name="l2", bufs=1) as l2:
                y2 = []
                for s in range(8):
                    t = l2.tile([64, 4096], bf16, tag=f"y2_{s}", name=f"y2b{s}")
                    nc.gpsimd.dma_start(t[:], yl_s[:] if s == 0 else yh2_s[s - 1])
                    y2.append(t)
                t2 = [
                    l2.tile([64, 64, 68], bf16, tag=f"t2_{p}", name=f"t2q{p}")
                    for p in range(4)
                ]
                m2 = [
                    l2.tile([64, 68, 128], bf16, tag=f"m2_{b}", name=f"m2q{b}")
                    for b in range(2)
                ]

                # D2: pair (y2[2p], y2[2p+1]) -> t2_p; 7 h2 per PSUM bank
                for p in range(4):
                    for g in range(10):
                        h2s = list(range(7 * g, min(7 * g + 7, 64)))
                        pst = ps.tile([128, 512], f32, tag="pb", name="psd2")
                        for j, h2 in enumerate(h2s):
                            nc.tensor.matmul(
                                pst[0:64, j * 68:(j + 1) * 68],
                                y2[2 * p][:, h2 * 64:(h2 + 1) * 64],
                                cbD2[0][:],
                                start=True, stop=False,
                            )
                            nc.tensor.matmul(
                                pst[0:64, j * 68:(j + 1) * 68],
                                y2[2 * p + 1][:, h2 * 64:(h2 + 1) * 64],
                                cbD2[1][:],
                                start=False, stop=True,
                            )
                        n = len(h2s)
                        cp_rr(t2[p][:, 7 * g:7 * g + n, :], pst[0:64, :n * 68])

                # W2: m2_b = Bc64_g0^T t2_b + Bc64_g1^T t2_{b+2}; 4 d per bank
                for b in range(2):
                    for g in range(17):
                        pst = ps.tile([128, 512], f32, tag="pb", name="psw2")
                        for j in range(4):
                            d = 4 * g + j
                            nc.tensor.matmul(
                                pst[0:64, j * 128:(j + 1) * 128],
                                t2[b][:, :, d:d + 1].rearrange("p h o -> p (h o)"),
                                cbW2[0][:],
                                start=True, stop=False,
                            )
                            nc.tensor.matmul(
                                pst[0:64, j * 128:(j + 1) * 128],
                                t2[b + 2][:, :, d:d + 1].rearrange("p h o -> p (h o)"),
                                cbW2[1][:],
                                start=False, stop=True,
                            )
                        cp_rr(m2[b][:, 4 * g:4 * g + 4, :], pst[0:64, :])

                # H2: lld = Bc64_g0^T m2_0 + Bc64_g1^T m2_1; 4 w per bank
                for g in range(32):
                    pst = ps.tile([128, 512], f32, tag="pb", name="psh2")
                    for j in range(4):
                        w = 4 * g + j
                        nc.tensor.matmul(
                            pst[0:68, j * 128:(j + 1) * 128],
                            m2[0][:, :, w:w + 1].rearrange("p d o -> p (d o)"),
                            cbW2[0][:],
                            start=True, stop=False,
                        )
                        nc.tensor.matmul(
                            pst[0:68, j * 128:(j + 1) * 128],
                            m2[1][:, :, w:w + 1].rearrange("p d o -> p (d o)"),
                            cbW2[1][:],
                            start=False, stop=True,
                        )
                    cp_rr(lld[:, 4 * g:4 * g + 4, :], pst[0:68, :])

            # ---------------- level 1 ----------------
            with (
                tc.tile_pool(name="tp", bufs=1) as tp,
                tc.tile_pool(name="y1p", bufs=1) as y1p,
                tc.tile_pool(name="m1p", bufs=1) as m1p,
                tc.tile_pool(name="sg", bufs=1) as sgp,
            ):
                tt = [
                    tp.tile([128, 128, 128], bf16, tag=f"t_{p}", name=f"t{p}")
                    for p in range(4)
                ]

                # D1: stream yh1 in h-chunks via gpsimd cast-DMA (f32 -> bf16)
                for hc in range(128 // H_CH):
                    y1t = []
                    for s in range(7):
                        t = y1p.tile(
                            [68, H_CH, 128], bf16, tag=f"y1_{s}", name=f"y1c{s}"
                        )
                        nc.gpsimd.dma_start(
                            t[:], yh1_s[s][:, hc * H_CH:(hc + 1) * H_CH, :]
                        )
                        y1t.append(t)
                    for hg in range(H_CH // 4):
                        psd = [
                            ps.tile([128, 512], f32, tag="pb", name=f"psd1_{p}")
                            for p in range(4)
                        ]
                        for hl in range(4 * hg, 4 * hg + 4):
                            h = hc * H_CH + hl
                            for p in range(4):
                                if p == 0:
                                    st_lo = lld[:, :, h:h + 1].rearrange(
                                        "p w o -> p (w o)"
                                    )
                                    st_hi = y1t[0][:, hl:hl + 1, :].rearrange(
                                        "p o w -> p (o w)"
                                    )
                                else:
                                    st_lo = y1t[2 * p - 1][:, hl:hl + 1, :].rearrange(
                                        "p o w -> p (o w)"
                                    )
                                    st_hi = y1t[2 * p][:, hl:hl + 1, :].rearrange(
                                        "p o w -> p (o w)"
                                    )
                                col = (hl % 4) * 128
                                nc.tensor.matmul(
                                    psd[p][:, col:col + 128],
                                    st_lo, cbD1[0][:],
                                    start=True, stop=False,
                                )
                                nc.tensor.matmul(
                                    psd[p][:, col:col + 128],
                                    st_hi, cbD1[1][:],
                                    start=False, stop=True,
                                )
                        h0 = hc * H_CH + 4 * hg
                        for p in range(4):
                            cp_rr(tt[p][:, h0:h0 + 4, :], psd[p][:])

                # W1 + H1 per d-chunk
                for dc in range(128 // DC):
                    m1 = [
                        m1p.tile(
                            [128, DC, 256], bf16, tag=f"m1_{b}", bufs=2,
                            name=f"m1{b}",
                        )
                        for b in range(2)
                    ]
                    for j in range(DC // 2):
                        for b in range(2):
                            pst = ps.tile([128, 512], f32, tag="pb", name="psw1")
                            for k in range(2):
                                d = dc * DC + j * 2 + k
                                nc.tensor.matmul(
                                    pst[:, k * 256:(k + 1) * 256],
                                    tt[b][:, :, d:d + 1].rearrange("p h o -> p (h o)"),
                                    cbc1[0][:],
                                    start=True, stop=False,
                                )
                                nc.tensor.matmul(
                                    pst[:, k * 256:(k + 1) * 256],
                                    tt[b + 2][:, :, d:d + 1].rearrange(
                                        "p h o -> p (h o)"
                                    ),
                                    cbc1[1][:],
                                    start=False, stop=True,
                                )
                            cp_rr(m1[b][:, j * 2:j * 2 + 2, :], pst[:])
                    m1f = [m1[b][:].rearrange("p d w -> p (d w)") for b in range(2)]
                    for ht in range(2):
                        for sg2 in range(DC * 256 // 1024):
                            stage = sgp.tile(
                                [128, 1024], f32, tag="stage", bufs=3,
                                name="stage",
                            )
                            for jj in range(2):
                                jt = sg2 * 2 + jj
                                pst = ps.tile([128, 512], f32, tag="pb", name="psh1")
                                nc.tensor.matmul(
                                    pst[:],
                                    cbc1[0][:, ht * 128:(ht + 1) * 128],
                                    m1f[0][:, jt * 512:(jt + 1) * 512],
                                    start=True, stop=False,
                                )
                                nc.tensor.matmul(
                                    pst[:],
                                    cbc1[1][:, ht * 128:(ht + 1) * 128],
                                    m1f[1][:, jt * 512:(jt + 1) * 512],
                                    start=False, stop=True,
                                )
                                hcp_rr(
                                    stage[:, jj * 512:(jj + 1) * 512], pst[:]
                                )
                            d0 = dc * DC + sg2 * 4
                            dst = out[d0:d0 + 4, ht * 128:(ht + 1) * 128, :]
                            dma_rr(
                                dst.rearrange("d h w -> h d w"),
                                stage[:].rearrange("p (d w) -> p d w", d=4),
                            )

    nc.finalize()
    return nc


def _get_nc():
    if "nc" not in _CACHE:
        _CACHE["nc"] = _build_nc()
    return _CACHE["nc"]


def make_in_maps(yl, yh1, yh2, g0, g1):
    g0 = np.asarray(g0, np.float32)
    g1 = np.asarray(g1, np.float32)
    yl = np.asarray(yl)
    yh1 = np.asarray(yh1)
    yh2 = np.asarray(yh2)
    B64 = [_band_cyclic(g0, 64), _band_cyclic(g1, 64)]
    B128 = [_band_cyclic(g0, 128), _band_cyclic(g1, 128)]
    bW2 = np.stack([B64[0], B64[1]]).astype(BF)
    bc1 = np.stack([B128[0], B128[1]]).astype(BF)
    in_maps = []
    for c in range(8):
        b, dh = c // 2, c % 2
        W1 = [(64 * dh - 2 + t) % 128 for t in range(68)]
        bD2 = np.stack([B64[0][:, W1], B64[1][:, W1]]).astype(BF)
        blk = slice(128 * dh, 128 * dh + 128)
        bD1 = np.stack([B128[0][W1, blk], B128[1][W1, blk]]).astype(BF)
        d0 = (64 * dh - 2) % 128
        n1 = min(68, 128 - d0)
        yh1w = np.concatenate(
            [yh1[b, :, d0:d0 + n1], yh1[b, :, :68 - n1]], axis=1
        )
        in_maps.append({
            "yl_s": np.ascontiguousarray(yl[b, 0].reshape(64, 4096)),
            "yh2_s": np.ascontiguousarray(yh2[b].reshape(7, 64, 4096)),
            "yh1_s": yh1w,
            "bD2": bD2,
            "bW2": bW2,
            "bD1": bD1,
            "bc1": bc1,
        })
    return in_maps


def assemble(results):
    out = np.empty((4, 1, 256, 256, 256), np.float32)
    for c in range(8):
        b, dh = c // 2, c % 2
        out[b, 0, 128 * dh:128 * (dh + 1)] = results[c]["out"]
    return out


def kernel(yl, yh1, yh2, g0, g1):
    from concourse.bass_utils import run_bass_kernel_spmd

    nc = _get_nc()
    in_maps = make_in_maps(yl, yh1, yh2, g0, g1)
    res = run_bass_kernel_spmd(nc, in_maps, list(range(8)))
    return assemble(res.results)


# revision 3
# speedup vs baseline: 1.4757x; 1.4757x over previous
"""Inverse 3D DWT (db4, 2 levels) Trainium2 Bass kernel, v2.

Sharding: 8 cores = 4 batch x 2 D-halves. Core (b, dh) computes the
contiguous output slab out[b, 0, 128*dh:128*(dh+1), :, :] (128x256x256).

Per level the three 1D synthesis stages run in the order D -> W -> H with
matmul orientations chosen so every stage's output lands with the next
stage's contraction dim on partitions: no PE transposes and no DRAM
round-trips. All intermediates stay in SBUF (bf16); PSUM accumulates fp32.

  level-2 (64^3 -> d-window 68 x 128 x 128):
    D: stat = input [64,64w] per h2,  mov = windowed band [64,68]   (lo+hi acc)
    W: stat = t2[:, :, d] (strided),  mov = band [64,128]           (lo+hi acc)
    H: stat = m2[:, :, w] (strided),  mov = band [64,128]           (lo+hi acc)
  level-1 (68 x 128 x 128 -> 128 x 256 x 256):
    D: stat = subband [68, 128w] per h, mov = windowed band [68,128]
       (pairs 1-3 first — they don't need level-2's output, so they
        overlap the level-2 tail; the lld pair runs as a second sweep)
    W: stat = t[:, :, d] (strided),  mov = full band [128,256]
    H: stat = band slice [128,128],  mov = m1 [128, (d,w)]

yh1 streams in h-chunks through gpsimd SWDGE cast-DMAs (f32->bf16 during
the transfer) so no engine cast copies are needed; PSUM evacuation
alternates Act/DVE (gpsimd cannot read PSUM on this HW).
"""
import sys
import numpy as np
import ml_dtypes

sys.path.insert(0, "/opt/trn_rl_repo")

_CACHE = {}
BF = ml_dtypes.bfloat16


def _band_cyclic(w, n):
    M = np.zeros((n, 2 * n), np.float32)
    for j in range(2 * n):
        k0 = (j + 3) % 2
        base = (j + 3) // 2
        for s in range(4):
            M[(base - s) % n, j] += w[k0 + 2 * s]
    return M


def _build_nc():
    import concourse.bass as bass  # noqa: F401
    import concourse.tile as tile
    import concourse.mybir as mybir
    from concourse import bacc

    f32 = mybir.dt.float32
    bf16 = mybir.dt.bfloat16

    nc = bacc.Bacc("TRN2", debug=False, num_devices=8)

    yl_s = nc.dram_tensor("yl_s", [64, 4096], f32, kind="ExternalInput")
    yh2_s = nc.dram_tensor("yh2_s", [7, 64, 4096], f32, kind="ExternalInput")
    yh1_s = nc.dram_tensor("yh1_s", [7, 68, 128, 128], f32, kind="ExternalInput")
    bD2 = nc.dram_tensor("bD2", [2, 64, 68], bf16, kind="ExternalInput")
    bW2 = nc.dram_tensor("bW2", [2, 64, 128], bf16, kind="ExternalInput")
    bD1 = nc.dram_tensor("bD1", [2, 68, 128], bf16, kind="ExternalInput")
    bc1 = nc.dram_tensor("bc1", [2, 128, 256], bf16, kind="ExternalInput")
    out = nc.dram_tensor("out", [128, 256, 256], f32, kind="ExternalOutput")

    H_CH = 8      # level-1 D h-chunk
    DC = 8        # level-1 W/H d-chunk

    # round-robin copy engines: PSUM evacuation on Act/DVE, casts mostly gpsimd
    def make_rr(pattern):
        state = [0]

        def cp(dst, src):
            e = pattern[state[0] % len(pattern)]
            state[0] += 1
            if e == 0:
                nc.scalar.copy(dst, src)
            elif e == 1:
                nc.vector.tensor_copy(dst, src)
            else:
                nc.gpsimd.tensor_copy(dst, src)

        return cp

    cp_rr = make_rr([0, 1])
    hcp_rr = make_rr([1, 0])

    # round-robin DMA issue across the two HWDGE queues (SP, Act)
    def make_dma_rr():
        state = [0]

        def dma(dst, src):
            e = state[0] % 2
            state[0] += 1
            (nc.sync if e == 0 else nc.scalar).dma_start(dst, src)

        return dma

    dma_rr = make_dma_rr()

    with tile.TileContext(nc) as tc:
        with (
            tc.tile_pool(name="pp", bufs=1) as pp,
            tc.tile_pool(name="ps", bufs=8, space="PSUM") as ps,
        ):
            # persistent tiles + constants
            lld = pp.tile([68, 128, 128], bf16, tag="lld")
            cbD2, cbW2, cbD1, cbc1 = [], [], [], []
            for s in range(2):
                tmp = pp.tile([64, 68], bf16, tag=f"cbD2_{s}", name=f"cbD2{s}")
                nc.sync.dma_start(tmp[:], bD2[s])
                cbD2.append(tmp)
                tmp = pp.tile([64, 128], bf16, tag=f"cbW2_{s}", name=f"cbW2{s}")
                nc.sync.dma_start(tmp[:], bW2[s])
                cbW2.append(tmp)
                tmp = pp.tile([68, 128], bf16, tag=f"cbD1_{s}", name=f"cbD1{s}")
                nc.sync.dma_start(tmp[:], bD1[s])
                cbD1.append(tmp)
                tmp = pp.tile([128, 256], bf16, tag=f"cbc1_{s}", name=f"cbc1{s}")
                nc.sync.dma_start(tmp[:], bc1[s])
                cbc1.append(tmp)

            # ---------------- level 2 ----------------
            with tc.tile_pool(name="l2", bufs=1) as l2:
                y2 = []
                for s in range(8):
                    t = l2.tile([64, 4096], bf16, tag=f"y2_{s}", name=f"y2b{s}")
                    nc.gpsimd.dma_start(t[:], yl_s[:] if s == 0 else yh2_s[s - 1])
                    y2.append(t)
                t2 = [
                    l2.tile([64, 64, 68], bf16, tag=f"t2_{p}", name=f"t2q{p}")
                    for p in range(4)
                ]
                m2 = [
                    l2.tile([64, 68, 128], bf16, tag=f"m2_{b}", name=f"m2q{b}")
                    for b in range(2)
                ]

                # D2: pair (y2[2p], y2[2p+1]) -> t2_p; 7 h2 per PSUM bank
                for p in range(4):
                    for g in range(10):
                        h2s = list(range(7 * g, min(7 * g + 7, 64)))
                        pst = ps.tile([128, 512], f32, tag="pb", name="psd2")
                        for j, h2 in enumerate(h2s):
                            nc.tensor.matmul(
                                pst[0:64, j * 68:(j + 1) * 68],
                                y2[2 * p][:, h2 * 64:(h2 + 1) * 64],
                                cbD2[0][:],
                                start=True, stop=False,
                            )
                            nc.tensor.matmul(
                                pst[0:64, j * 68:(j + 1) * 68],
                                y2[2 * p + 1][:, h2 * 64:(h2 + 1) * 64],
                                cbD2[1][:],
                                start=False, stop=True,
                            )
                        n = len(h2s)
                        cp_rr(t2[p][:, 7 * g:7 * g + n, :], pst[0:64, :n * 68])

                # W2: m2_b = Bc64_g0^T t2_b + Bc64_g1^T t2_{b+2}; 4 d per bank
                for b in range(2):
                    for g in range(17):
                        pst = ps.tile([128, 512], f32, tag="pb", name="psw2")
                        for j in range(4):
                            d = 4 * g + j
                            nc.tensor.matmul(
                                pst[0:64, j * 128:(j + 1) * 128],
                                t2[b][:, :, d:d + 1].rearrange("p h o -> p (h o)"),
                                cbW2[0][:],
                                start=True, stop=False,
                            )
                            nc.tensor.matmul(
                                pst[0:64, j * 128:(j + 1) * 128],
                                t2[b + 2][:, :, d:d + 1].rearrange("p h o -> p (h o)"),
                                cbW2[1][:],
                                start=False, stop=True,
                            )
                        cp_rr(m2[b][:, 4 * g:4 * g + 4, :], pst[0:64, :])

                # H2: lld = Bc64_g0^T m2_0 + Bc64_g1^T m2_1; 4 w per bank
                for g in range(32):
                    pst = ps.tile([128, 512], f32, tag="pb", name="psh2")
                    for j in range(4):
                        w = 4 * g + j
                        nc.tensor.matmul(
                            pst[0:68, j * 128:(j + 1) * 128],
                            m2[0][:, :, w:w + 1].rearrange("p d o -> p (d o)"),
                            cbW2[0][:],
                            start=True, stop=False,
                        )
                        nc.tensor.matmul(
                            pst[0:68, j * 128:(j + 1) * 128],
                            m2[1][:, :, w:w + 1].rearrange("p d o -> p (d o)"),
                            cbW2[1][:],
                            start=False, stop=True,
                        )
                    cp_rr(lld[:, 4 * g:4 * g + 4, :], pst[0:68, :])

            # ---------------- level 1 ----------------
            with (
                tc.tile_pool(name="tp", bufs=1) as tp,
                tc.tile_pool(name="y1p", bufs=1) as y1p,
                tc.tile_pool(name="m1p", bufs=1) as m1p,
                tc.tile_pool(name="sg", bufs=1) as sgp,
            ):
                tt = [
                    tp.tile([128, 128, 128], bf16, tag=f"t_{p}", name=f"t{p}")
                    for p in range(4)
                ]

                # D1 sweep A: pairs 1-3 (yh1 only) — independent of level-2,
                # so these overlap the level-2 tail. Pair 0 (lld) runs after.
                for hc in range(128 // H_CH):
                    y1t = {}
                    for s in range(1, 7):
                        t = y1p.tile(
                            [68, H_CH, 128], bf16, tag=f"y1_{s}", name=f"y1c{s}"
                        )
                        nc.gpsimd.dma_start(
                            t[:], yh1_s[s][:, hc * H_CH:(hc + 1) * H_CH, :]
                        )
                        y1t[s] = t
                    for hg in range(H_CH // 4):
                        psd = [
                            ps.tile([128, 512], f32, tag="pb", name=f"psd1_{p}")
                            for p in range(3)
                        ]
                        for hl in range(4 * hg, 4 * hg + 4):
                            col = (hl % 4) * 128
                            for p in range(1, 4):
                                st_lo = y1t[2 * p - 1][:, hl:hl + 1, :].rearrange(
                                    "p o w -> p (o w)"
                                )
                                st_hi = y1t[2 * p][:, hl:hl + 1, :].rearrange(
                                    "p o w -> p (o w)"
                                )
                                nc.tensor.matmul(
                                    psd[p - 1][:, col:col + 128],
                                    st_lo, cbD1[0][:],
                                    start=True, stop=False,
                                )
                                nc.tensor.matmul(
                                    psd[p - 1][:, col:col + 128],
                                    st_hi, cbD1[1][:],
                                    start=False, stop=True,
                                )
                        h0 = hc * H_CH + 4 * hg
                        for p in range(1, 4):
                            cp_rr(tt[p][:, h0:h0 + 4, :], psd[p - 1][:])

                # D1 sweep B: pair 0 = (lld, yh1[0])
                for hc in range(128 // H_CH):
                    t0 = y1p.tile(
                        [68, H_CH, 128], bf16, tag="y1_0", name="y1c0"
                    )
                    nc.gpsimd.dma_start(
                        t0[:], yh1_s[0][:, hc * H_CH:(hc + 1) * H_CH, :]
                    )
                    for hg in range(H_CH // 4):
                        psd0 = ps.tile([128, 512], f32, tag="pb", name="psd1_0")
                        for hl in range(4 * hg, 4 * hg + 4):
                            h = hc * H_CH + hl
                            col = (hl % 4) * 128
                            nc.tensor.matmul(
                                psd0[:, col:col + 128],
                                lld[:, :, h:h + 1].rearrange("p w o -> p (w o)"),
                                cbD1[0][:],
                                start=True, stop=False,
                            )
                            nc.tensor.matmul(
                                psd0[:, col:col + 128],
                                t0[:, hl:hl + 1, :].rearrange("p o w -> p (o w)"),
                                cbD1[1][:],
                                start=False, stop=True,
                            )
                        h0 = hc * H_CH + 4 * hg
                        cp_rr(tt[0][:, h0:h0 + 4, :], psd0[:])

                # W1 + H1 per d-chunk
                for dc in range(128 // DC):
                    m1 = [
                        m1p.tile(
                            [128, DC, 256], bf16, tag=f"m1_{b}", bufs=2,
                            name=f"m1{b}",
                        )
                        for b in range(2)
                    ]
                    for j in range(DC // 2):
                        for b in range(2):
                            pst = ps.tile([128, 512], f32, tag="pb", name="psw1")
                            for k in range(2):
                                d = dc * DC + j * 2 + k
                                nc.tensor.matmul(
                                    pst[:, k * 256:(k + 1) * 256],
                                    tt[b][:, :, d:d + 1].rearrange("p h o -> p (h o)"),
                                    cbc1[0][:],
                                    start=True, stop=False,
                                )
                                nc.tensor.matmul(
                                    pst[:, k * 256:(k + 1) * 256],
                                    tt[b + 2][:, :, d:d + 1].rearrange(
                                        "p h o -> p (h o)"
                                    ),
                                    cbc1[1][:],
                                    start=False, stop=True,
                                )
                            cp_rr(m1[b][:, j * 2:j * 2 + 2, :], pst[:])
                    m1f = [m1[b][:].rearrange("p d w -> p (d w)") for b in range(2)]
                    for ht in range(2):
                        for sg2 in range(DC * 256 // 1024):
                            stage = sgp.tile(
                                [128, 1024], f32, tag="stage", bufs=3,
                                name="stage",
                            )
                            for jj in range(2):
                                jt = sg2 * 2 + jj
                                pst = ps.tile([128, 512], f32, tag="pb", name="psh1")
                                nc.tensor.matmul(
                                    pst[:],
                                    cbc1[0][:, ht * 128:(ht + 1) * 128],
                                    m1f[0][:, jt * 512:(jt + 1) * 512],
                                    start=True, stop=False,
                                )
                                nc.tensor.matmul(
                                    pst[:],
                                    cbc1[1][:, ht * 128:(ht + 1) * 128],
                                    m1f[1][:, jt * 512:(jt + 1) * 512],
                                    start=False, stop=True,
                                )
                                hcp_rr(
                                    stage[:, jj * 512:(jj + 1) * 512], pst[:]
                                )
                            d0 = dc * DC + sg2 * 4
                            dst = out[d0:d0 + 4, ht * 128:(ht + 1) * 128, :]
                            dma_rr(
                                dst.rearrange("d h w -> h d w"),
                                stage[:].rearrange("p (d w) -> p d w", d=4),
                            )

    nc.finalize()
    return nc


def _get_nc():
    if "nc" not in _CACHE:
        _CACHE["nc"] = _build_nc()
    return _CACHE["nc"]


def make_in_maps(yl, yh1, yh2, g0, g1):
    g0 = np.asarray(g0, np.float32)
    g1 = np.asarray(g1, np.float32)
    yl = np.asarray(yl)
    yh1 = np.asarray(yh1)
    yh2 = np.asarray(yh2)
    B64 = [_band_cyclic(g0, 64), _band_cyclic(g1, 64)]
    B128 = [_band_cyclic(g0, 128), _band_cyclic(g1, 128)]
    bW2 = np.stack([B64[0], B64[1]]).astype(BF)
    bc1 = np.stack([B128[0], B128[1]]).astype(BF)
    in_maps = []
    for c in range(8):
        b, dh = c // 2, c % 2
        W1 = [(64 * dh - 2 + t) % 128 for t in range(68)]
        bD2 = np.stack([B64[0][:, W1], B64[1][:, W1]]).astype(BF)
        blk = slice(128 * dh, 128 * dh + 128)
        bD1 = np.stack([B128[0][W1, blk], B128[1][W1, blk]]).astype(BF)
        d0 = (64 * dh - 2) % 128
        n1 = min(68, 128 - d0)
        yh1w = np.concatenate(
            [yh1[b, :, d0:d0 + n1], yh1[b, :, :68 - n1]], axis=1
        )
        in_maps.append({
            "yl_s": np.ascontiguousarray(yl[b, 0].reshape(64, 4096)),
            "yh2_s": np.ascontiguousarray(yh2[b].reshape(7, 64, 4096)),
            "yh1_s": yh1w,
            "bD2": bD2,
            "bW2": bW2,
            "bD1": bD1,
            "bc1": bc1,
        })
    return in_maps


def assemble(results):
    out = np.empty((4, 1, 256, 256, 256), np.float32)
    for c in range(8):
        b, dh = c // 2, c % 2
        out[b, 0, 128 * dh:128 * (dh + 1)] = results[c]["out"]
    return out


def kernel(yl, yh1, yh2, g0, g1):
    from concourse.bass_utils import run_bass_kernel_spmd

    nc = _get_nc()
    in_maps = make_in_maps(yl, yh1, yh2, g0, g1)
    res = run_bass_kernel_spmd(nc, in_maps, list(range(8)))
    return assemble(res.results)
